# revision 4
# baseline (speedup 1.0000x reference)
"""Trainium2 Bass kernel for NLBlock (non-local block, embedded gaussian, 1D).

Reference computation (B=4, C=512, CI=256, T=4096):
    g/theta/phi = 1x1 conv of x          (B,CI,T)
    f = theta^T @ phi                    (B,T,T)
    attn = softmax(f, axis=-1)
    y = attn @ g^T                       (B,CI,T)
    w_y = W_z @ y + b_z                  (B,C,T)
    BN(w_y) * gamma + beta + x           -> (B,C,T,1)

Sharding: 8 cores = (batch b, query-half h).  Each core holds the full
key/value sequence for its batch (phi, g over all T) and computes
queries for its half (T/2 = 2048).  BatchNorm statistics are combined
with a tiny AllReduce ([128,8] floats) across all 8 cores.

All matmuls run in float32r (TF32-like, full PE rate).
"""
import sys
import numpy as np

sys.path.insert(0, '/opt/trn_rl_repo')

B, C, CI, T = 4, 512, 256, 4096
NQ = T // 2          # queries per core
N_CORES = 8
BN_EPS = 1e-5

_COMPILED = None


def _build():
    import concourse.bass as bass
    import concourse.tile as tile
    from concourse import bacc, mybir
    from contextlib import ExitStack

    f32 = mybir.dt.float32
    f32r = mybir.dt.float32r
    AF = mybir.ActivationFunctionType
    AX = mybir.AxisListType

    nc = bacc.Bacc("TRN2", target_bir_lowering=False, debug=False,
                   num_devices=N_CORES)

    # ---- per-core DRAM I/O ----------------------------------------------
    # x for keys: full batch-row, c-chunk layout [p, kc, t]
    x_d = nc.dram_tensor("x", [128, 4, T], f32r, kind="ExternalInput")
    # x for queries: this core's half  [p, kc, q]
    xq_d = nc.dram_tensor("xq", [128, 4, NQ], f32r, kind="ExternalInput")
    wth_d = nc.dram_tensor("wthT", [128, 4, CI], f32r, kind="ExternalInput")
    wph_d = nc.dram_tensor("wphT", [128, 4, CI], f32r, kind="ExternalInput")
    wg_d = nc.dram_tensor("wgT", [128, 4, CI], f32r, kind="ExternalInput")
    wz_d = nc.dram_tensor("wzT", [128, 2, C], f32r, kind="ExternalInput")
    bth_d = nc.dram_tensor("bth", [128, 2], f32, kind="ExternalInput")
    bph_d = nc.dram_tensor("bph", [128, 2], f32, kind="ExternalInput")
    bzp_d = nc.dram_tensor("bzp", [128, 4], f32, kind="ExternalInput")
    gam_d = nc.dram_tensor("gam", [128, 4], f32, kind="ExternalInput")
    bet_d = nc.dram_tensor("bet", [128, 4], f32, kind="ExternalInput")
    id_d = nc.dram_tensor("ident", [128, 128], f32r, kind="ExternalInput")
    z_d = nc.dram_tensor("z", [128, 4, NQ], f32, kind="ExternalOutput")
    # internal scratch
    wy_d = nc.dram_tensor("wy_scratch", [128, 4, NQ], f32)
    cc_in = nc.dram_tensor("cc_in", [128, 8], f32)
    cc_out = nc.dram_tensor("cc_out", [128, 8], f32, addr_space="Shared")

    NTB = T // 512       # 8 t-blocks for keys
    NQB = NQ // 512      # 4 t-blocks for queries
    NQT = NQ // 128      # 16 query tiles

    with tile.TileContext(nc) as tc:
        with ExitStack() as ctx:
            ep = ctx.enter_context
            # ------- SBUF pools -------
            wpool = ep(tc.tile_pool(name="weights", bufs=1))
            xpool = ep(tc.tile_pool(name="xin", bufs=3))
            phip = ep(tc.tile_pool(name="phi", bufs=1))
            thp = ep(tc.tile_pool(name="theta", bufs=1))
            gtp = ep(tc.tile_pool(name="gt", bufs=1))
            ppool = ep(tc.tile_pool(name="pmat", bufs=2))
            ptp = ep(tc.tile_pool(name="ptrans", bufs=4))
            ytp = ep(tc.tile_pool(name="ytmp", bufs=4))
            ygp = ep(tc.tile_pool(name="ygrp", bufs=2))
            wyp = ep(tc.tile_pool(name="wyout", bufs=2))
            sqp = ep(tc.tile_pool(name="sq", bufs=2))
            smp = ep(tc.tile_pool(name="small", bufs=24))
            stp = ep(tc.tile_pool(name="stats", bufs=1))
            apl = ep(tc.tile_pool(name="apply", bufs=2))
            # ------- PSUM pools -------
            mmp = ep(tc.tile_pool(name="mm", bufs=4, space="PSUM"))
            trp = ep(tc.tile_pool(name="tr", bufs=2, space="PSUM"))
            yps = ep(tc.tile_pool(name="yp", bufs=2, space="PSUM"))

            # ------- load weights -------
            wth = wpool.tile([128, 4, CI], f32r)
            wph = wpool.tile([128, 4, CI], f32r)
            wg = wpool.tile([128, 4, CI], f32r)
            wz = wpool.tile([128, 2, C], f32r)
            bth = wpool.tile([128, 2], f32)
            bph = wpool.tile([128, 2], f32)
            bzp = wpool.tile([128, 4], f32)
            gam = wpool.tile([128, 4], f32)
            bet = wpool.tile([128, 4], f32)
            ident = wpool.tile([128, 128], f32r)
            for t_, d_ in ((wth, wth_d), (wph, wph_d), (wg, wg_d),
                           (wz, wz_d), (bth, bth_d), (bph, bph_d),
                           (bzp, bzp_d), (gam, gam_d), (bet, bet_d),
                           (ident, id_d)):
                nc.sync.dma_start(t_[:], d_[:])

            # ------- persistent activations -------
            phi = phip.tile([128, 2, T], f32r)       # [ci_p, m, s]
            th = thp.tile([128, 2, NQ], f32r)        # [ci_p, m, q]
            gt = gtp.tile([128, T // 128, CI], f32r)  # [s_p, j, ci]

            # ------- conv phase: phi + gT from x (keys) -------
            for tb in range(NTB):
                xt = xpool.tile([128, 4, 512], f32r, tag="xt")
                nc.sync.dma_start(xt[:], x_d[:, :, tb * 512:(tb + 1) * 512])
                for m in range(2):
                    ps = mmp.tile([128, 512], f32)
                    for kc in range(4):
                        nc.tensor.matmul(
                            ps[:], wph[:, kc, m * 128:(m + 1) * 128],
                            xt[:, kc, :], start=(kc == 0), stop=(kc == 3))
                    nc.scalar.activation(phi[:, m, tb * 512:(tb + 1) * 512],
                                         ps[:], AF.Identity,
                                         bias=bph[:, m:m + 1])
                for j in range(4):
                    ps2 = trp.tile([128, CI], f32, tag="tr")
                    for kc in range(4):
                        nc.tensor.matmul(
                            ps2[:], xt[:, kc, j * 128:(j + 1) * 128],
                            wg[:, kc, :], start=(kc == 0), stop=(kc == 3))
                    nc.any.tensor_copy(gt[:, tb * 4 + j, :], ps2[:])

            # ------- conv phase: theta from xq (queries) -------
            for tb in range(NQB):
                xt = xpool.tile([128, 4, 512], f32r, tag="xt")
                nc.sync.dma_start(xt[:], xq_d[:, :, tb * 512:(tb + 1) * 512])
                for m in range(2):
                    ps = mmp.tile([128, 512], f32)
                    for kc in range(4):
                        nc.tensor.matmul(
                            ps[:], wth[:, kc, m * 128:(m + 1) * 128],
                            xt[:, kc, :], start=(kc == 0), stop=(kc == 3))
                    nc.scalar.activation(th[:, m, tb * 512:(tb + 1) * 512],
                                         ps[:], AF.Identity,
                                         bias=bth[:, m:m + 1])

            # ------- attention + w_z -------
            s1acc = stp.tile([128, 4, 4], f32)
            s2acc = stp.tile([128, 4, 4], f32)
            ygrp = None
            for qt in range(NQT):
                if qt % 4 == 0:
                    ygrp = ygp.tile([128, 2, 512], f32r, tag="ygrp")
                P = ppool.tile([128, T], f32r, tag="P")
                mx8 = smp.tile([128, 8], f32, tag="mx8")
                l8 = smp.tile([128, 8], f32, tag="l8")
                mh = [None, None]
                # scores + per-half softmax
                for h in range(2):
                    blocks = []
                    for sb in range(4):
                        j = h * 4 + sb
                        ps = mmp.tile([128, 512], f32)
                        for m in range(2):
                            nc.tensor.matmul(
                                ps[:], th[:, m, qt * 128:(qt + 1) * 128],
                                phi[:, m, j * 512:(j + 1) * 512],
                                start=(m == 0), stop=(m == 1))
                        nc.vector.reduce_max(mx8[:, j:j + 1], ps[:], axis=AX.X)
                        blocks.append(ps)
                    mh_h = smp.tile([128, 1], f32, tag="mh")
                    nc.vector.reduce_max(mh_h[:], mx8[:, h * 4:(h + 1) * 4],
                                         axis=AX.X)
                    nm_h = smp.tile([128, 1], f32, tag="nm")
                    nc.vector.tensor_scalar_mul(nm_h[:], mh_h[:], -1.0)
                    mh[h] = mh_h
                    for sb in range(4):
                        j = h * 4 + sb
                        nc.scalar.activation(
                            P[:, j * 512:(j + 1) * 512], blocks[sb][:],
                            AF.Exp, bias=nm_h[:], accum_out=l8[:, j:j + 1])
                # merge halves
                mg = smp.tile([128, 1], f32, tag="mg")
                nc.vector.tensor_max(mg[:], mh[0][:], mh[1][:])
                nmg = smp.tile([128, 1], f32, tag="nmg")
                nc.vector.tensor_scalar_mul(nmg[:], mg[:], -1.0)
                ch = []
                for h in range(2):
                    c_h = smp.tile([128, 1], f32, tag="ch")
                    nc.scalar.activation(c_h[:], mh[h][:], AF.Exp,
                                         bias=nmg[:])
                    ch.append(c_h)
                lsum = smp.tile([128, 2], f32, tag="lsum")
                for h in range(2):
                    nc.vector.reduce_sum(lsum[:, h:h + 1],
                                         l8[:, h * 4:(h + 1) * 4], axis=AX.X)
                lt = smp.tile([128, 2], f32, tag="lt")
                for h in range(2):
                    nc.vector.tensor_mul(lt[:, h:h + 1], ch[h][:],
                                         lsum[:, h:h + 1])
                lg = smp.tile([128, 1], f32, tag="lg")
                nc.vector.tensor_add(lg[:], lt[:, 0:1], lt[:, 1:2])
                rg = smp.tile([128, 1], f32, tag="rg")
                nc.vector.reciprocal(rg[:], lg[:])
                rh = smp.tile([128, 2], f32, tag="rh")
                for h in range(2):
                    nc.vector.tensor_mul(rh[:, h:h + 1], ch[h][:], rg[:])
                # transpose P + y matmul
                y_ps = yps.tile([128, 512], f32)
                for h in range(2):
                    for jj in range(16):
                        j = h * 16 + jj
                        tr = trp.tile([128, 128], f32r, tag="tr")
                        nc.tensor.transpose(tr[:], P[:, j * 128:(j + 1) * 128],
                                            ident[:])
                        pt = ptp.tile([128, 128], f32r)
                        nc.any.tensor_copy(pt[:], tr[:])
                        nc.tensor.matmul(
                            y_ps[:, h * 256:(h + 1) * 256], pt[:],
                            gt[:, j, :], start=(jj == 0), stop=(jj == 15),
                            skip_group_check=True)
                # normalize + merge halves of y
                yt0 = ytp.tile([128, CI], f32, tag="yt")
                nc.scalar.activation(yt0[:], y_ps[:, 0:CI], AF.Identity,
                                     scale=rh[:, 0:1])
                yt1 = ytp.tile([128, CI], f32, tag="yt")
                nc.scalar.activation(yt1[:], y_ps[:, CI:2 * CI], AF.Identity,
                                     scale=rh[:, 1:2])
                ytile = ytp.tile([128, CI], f32r, tag="ytile")
                nc.vector.tensor_add(ytile[:], yt0[:], yt1[:])
                # transpose y tile into [ci, q] group buffer
                qq = qt % 4
                for ci in range(2):
                    tr = trp.tile([128, 128], f32r, tag="tr")
                    nc.tensor.transpose(tr[:], ytile[:, ci * 128:(ci + 1) * 128],
                                        ident[:])
                    nc.any.tensor_copy(ygrp[:, ci, qq * 128:(qq + 1) * 128],
                                       tr[:])
                # w_z conv per completed group of 512 queries
                if qt % 4 == 3:
                    g = qt // 4
                    for cc in range(4):
                        ps = mmp.tile([128, 512], f32)
                        for m in range(2):
                            nc.tensor.matmul(
                                ps[:], wz[:, m, cc * 128:(cc + 1) * 128],
                                ygrp[:, m, :], start=(m == 0), stop=(m == 1))
                        wyt = wyp.tile([128, 512], f32)
                        nc.scalar.activation(wyt[:], ps[:], AF.Identity,
                                             bias=bzp[:, cc:cc + 1],
                                             accum_out=s1acc[:, cc, g:g + 1])
                        sq = sqp.tile([128, 512], f32)
                        nc.scalar.activation(sq[:], wyt[:], AF.Square,
                                             accum_out=s2acc[:, cc, g:g + 1])
                        nc.sync.dma_start(wy_d[:, cc, g * 512:(g + 1) * 512],
                                          wyt[:])

            # ------- BN stats + collective -------
            stats = stp.tile([128, 8], f32)
            nc.vector.reduce_sum(stats[:, 0:4], s1acc[:], axis=AX.X)
            nc.vector.reduce_sum(stats[:, 4:8], s2acc[:], axis=AX.X)
            nc.sync.dma_start(cc_in[:, :], stats[:])
            nc.gpsimd.collective_compute(
                "AllReduce", mybir.AluOpType.add,
                replica_groups=[list(range(N_CORES))],
                ins=[cc_in.ap().opt()], outs=[cc_out.ap().opt()])
            stin = stp.tile([128, 8], f32)
            nc.sync.dma_start(stin[:], cc_out[:, :])
            inv_n = 1.0 / (B * T)
            mean = stp.tile([128, 4], f32)
            nc.vector.tensor_scalar_mul(mean[:], stin[:, 0:4], inv_n)
            ex2 = stp.tile([128, 4], f32)
            nc.vector.tensor_scalar_mul(ex2[:], stin[:, 4:8], inv_n)
            msq = stp.tile([128, 4], f32)
            nc.vector.tensor_mul(msq[:], mean[:], mean[:])
            var = stp.tile([128, 4], f32)
            nc.vector.tensor_sub(var[:], ex2[:], msq[:])
            vpe = stp.tile([128, 4], f32)
            nc.vector.tensor_scalar_add(vpe[:], var[:], BN_EPS)
            inv = stp.tile([128, 4], f32)
            nc.vector.reciprocal(inv[:], vpe[:])
            rstd = stp.tile([128, 4], f32)
            nc.scalar.sqrt(rstd[:], inv[:])
            a_t = stp.tile([128, 4], f32)
            nc.vector.tensor_mul(a_t[:], gam[:], rstd[:])
            ma = stp.tile([128, 4], f32)
            nc.vector.tensor_mul(ma[:], mean[:], a_t[:])
            bsh = stp.tile([128, 4], f32)
            nc.vector.tensor_sub(bsh[:], bet[:], ma[:])

            # ------- BN apply + residual + write out -------
            f32_ = mybir.dt.float32
            for cc in range(4):
                for qb in range(NQB):
                    wyr = apl.tile([128, 512], f32_, tag="wyr")
                    nc.sync.dma_start(wyr[:],
                                      wy_d[:, cc, qb * 512:(qb + 1) * 512])
                    xr = apl.tile([128, 512], f32_, tag="xr")
                    nc.sync.dma_start(
                        xr[:],
                        xq_d[:, cc, qb * 512:(qb + 1) * 512].bitcast(f32_))
                    t1 = apl.tile([128, 512], f32_, tag="t1")
                    nc.scalar.activation(t1[:], wyr[:], AF.Identity,
                                         scale=a_t[:, cc:cc + 1],
                                         bias=bsh[:, cc:cc + 1])
                    outt = apl.tile([128, 512], f32_, tag="outt")
                    nc.vector.tensor_add(outt[:], t1[:], xr[:])
                    nc.sync.dma_start(z_d[:, cc, qb * 512:(qb + 1) * 512],
                                      outt[:])

    nc.compile()
    return nc


def _get_compiled():
    global _COMPILED
    if _COMPILED is None:
        _COMPILED = _build()
    return _COMPILED


def _prep_inputs(x, W_g, b_g, W_theta, b_theta, W_phi, b_phi, W_z, b_z,
                 gamma, beta):
    """Host-side slicing/layout.  Returns list of per-core input dicts."""
    def cmaj(w):                       # (CI, C) -> [128, C//128, CI]
        return np.ascontiguousarray(
            w.T.reshape(C // 128, 128, w.shape[0]).transpose(1, 0, 2))

    wth = cmaj(W_theta)
    wph = cmaj(W_phi)
    wg = cmaj(W_g)
    wz = np.ascontiguousarray(
        W_z.T.reshape(2, 128, C).transpose(1, 0, 2))      # [128,2,512]
    bth = np.ascontiguousarray(b_theta.reshape(2, 128).T)
    bph = np.ascontiguousarray(b_phi.reshape(2, 128).T)
    bzp = np.ascontiguousarray((b_z + W_z @ b_g).reshape(4, 128).T)
    gam = np.ascontiguousarray(gamma.reshape(4, 128).T)
    bet = np.ascontiguousarray(beta.reshape(4, 128).T)
    ident = np.eye(128, dtype=np.float32)

    in_maps = []
    for k in range(N_CORES):
        b = k // 2
        q0 = (k % 2) * NQ
        xb = np.ascontiguousarray(
            x[b].reshape(4, 128, T).transpose(1, 0, 2))   # [128,4,T]
        xq = np.ascontiguousarray(xb[:, :, q0:q0 + NQ])
        in_maps.append({
            "x": xb, "xq": xq, "wthT": wth, "wphT": wph, "wgT": wg,
            "wzT": wz, "bth": bth, "bph": bph, "bzp": bzp, "gam": gam,
            "bet": bet, "ident": ident,
        })
    return in_maps


def kernel(x, W_g, b_g, W_theta, b_theta, W_phi, b_phi, W_z, b_z,
           gamma, beta, mesh=None, _trace=False):
    from concourse import bass_utils
    x = np.asarray(x, dtype=np.float32)
    args = [np.asarray(a, dtype=np.float32) for a in
            (W_g, b_g, W_theta, b_theta, W_phi, b_phi, W_z, b_z, gamma, beta)]
    nc = _get_compiled()
    in_maps = _prep_inputs(x, *args)
    res = bass_utils.run_bass_kernel_spmd(
        nc, in_maps, core_ids=list(range(N_CORES)), trace=_trace)
    out = np.empty((B, C, T), dtype=np.float32)
    for k in range(N_CORES):
        b = k // 2
        q0 = (k % 2) * NQ
        zc = res.results[k]["z"]                       # [128, 4, NQ]
        out[b, :, q0:q0 + NQ] = zc.transpose(1, 0, 2).reshape(C, NQ)
    if _trace:
        kernel._last_exec_time_ns = res.exec_time_ns
    return out[..., None]


# revision 13
# speedup vs baseline: 1.3163x; 1.3163x over previous
"""Trainium2 Bass kernel for NLBlock (non-local block, embedded gaussian, 1D).

Reference computation (B=4, C=512, CI=256, T=4096):
    g/theta/phi = 1x1 conv of x          (B,CI,T)
    f = theta^T @ phi                    (B,T,T)
    attn = softmax(f, axis=-1)
    y = attn @ g^T                       (B,CI,T)
    w_y = W_z @ y + b_z                  (B,C,T)
    BN(w_y) * gamma + beta + x           -> (B,C,T,1)

Sharding: 8 cores = (batch b, query-half).  Each core holds the full
key/value sequence for its batch (phi, g over all T) and computes
queries for its half (T/2 = 2048).  BatchNorm statistics are combined
with a tiny AllReduce ([128,8] floats) across all 8 cores.

Numerics: matmul operands in fp16 (10-bit mantissa, fp32 accumulate);
softmax scores reduced with per-row max; softmax normalization (exp
correction c_h and 1/l) is folded into the P-transpose by using a
diagonal matrix diag(r_h) instead of the identity as the transpose
operand.  b_g is folded into b_z (attn rows sum to 1):
b_z' = b_z + W_z @ b_g.
"""
import sys
import numpy as np

sys.path.insert(0, '/opt/trn_rl_repo')

B, C, CI, T = 4, 512, 256, 4096
NQ = T // 2          # queries per core
N_CORES = 8
BN_EPS = 1e-5

_COMPILED = None


def _build():
    import concourse.bass as bass
    import concourse.tile as tile
    from concourse import bacc, mybir
    from contextlib import ExitStack

    f32 = mybir.dt.float32
    f16 = mybir.dt.float16
    AF = mybir.ActivationFunctionType
    AX = mybir.AxisListType

    nc = bacc.Bacc("TRN2", target_bir_lowering=False, debug=False,
                   num_devices=N_CORES)

    # ---- per-core DRAM I/O ----------------------------------------------
    x_d = nc.dram_tensor("x", [128, 4, T], f16, kind="ExternalInput")
    xq_d = nc.dram_tensor("xq", [128, 4, NQ], f16, kind="ExternalInput")
    xq32_d = nc.dram_tensor("xq32", [128, 4, NQ], f32, kind="ExternalInput")
    wth_d = nc.dram_tensor("wthT", [128, 4, CI], f16, kind="ExternalInput")
    wph_d = nc.dram_tensor("wphT", [128, 4, CI], f16, kind="ExternalInput")
    wg_d = nc.dram_tensor("wgT", [128, 4, CI], f16, kind="ExternalInput")
    wz_d = nc.dram_tensor("wzT", [128, 2, C], f16, kind="ExternalInput")
    bth_d = nc.dram_tensor("bth", [128, 2], f32, kind="ExternalInput")
    bph_d = nc.dram_tensor("bph", [128, 2], f32, kind="ExternalInput")
    bzp_d = nc.dram_tensor("bzp", [128, 4], f32, kind="ExternalInput")
    gam_d = nc.dram_tensor("gam", [128, 4], f32, kind="ExternalInput")
    bet_d = nc.dram_tensor("bet", [128, 4], f32, kind="ExternalInput")
    id_d = nc.dram_tensor("ident", [128, 128], f16, kind="ExternalInput")
    z_d = nc.dram_tensor("z", [128, 4, NQ], f32, kind="ExternalOutput")
    cc_in = nc.dram_tensor("cc_in", [128, 8], f32)
    cc_out = nc.dram_tensor("cc_out", [128, 8], f32, addr_space="Shared")

    NTB = T // 512       # 8 key t-blocks
    NQB = NQ // 512      # 4 query 512-blocks
    NQT = NQ // 128      # 16 query tiles
    NHG = NQT // 2       # 8 half-groups (2 q-tiles each)

    with tile.TileContext(nc) as tc:
        with ExitStack() as ctx:
            ep = ctx.enter_context
            # ------- SBUF pools -------
            wpool = ep(tc.tile_pool(name="weights", bufs=1))
            xpool = ep(tc.tile_pool(name="xin", bufs=3))
            phip = ep(tc.tile_pool(name="phi", bufs=1))
            thp = ep(tc.tile_pool(name="theta", bufs=1))
            gtp = ep(tc.tile_pool(name="gt", bufs=1))
            ppool = ep(tc.tile_pool(name="pmat", bufs=5))
            ptp = ep(tc.tile_pool(name="ptw", bufs=3))
            ygp = ep(tc.tile_pool(name="ygrp", bufs=2))
            wyp = ep(tc.tile_pool(name="wy", bufs=1))
            sqp = ep(tc.tile_pool(name="sq", bufs=2))
            smp = ep(tc.tile_pool(name="small", bufs=24))
            stp = ep(tc.tile_pool(name="stats", bufs=1))
            apl = ep(tc.tile_pool(name="apply", bufs=3))
            # ------- PSUM pools -------
            mmp = ep(tc.tile_pool(name="mm", bufs=4, space="PSUM"))
            trp = ep(tc.tile_pool(name="tr", bufs=2, space="PSUM"))
            yps = ep(tc.tile_pool(name="yp", bufs=1, space="PSUM"))

            # ------- load inputs -------
            wth = wpool.tile([128, 4, CI], f16)
            wph = wpool.tile([128, 4, CI], f16)
            wg = wpool.tile([128, 4, CI], f16)
            wz = wpool.tile([128, 2, C], f16)
            bth = wpool.tile([128, 2], f32)
            bph = wpool.tile([128, 2], f32)
            bzp = wpool.tile([128, 4], f32)
            gam = wpool.tile([128, 4], f32)
            bet = wpool.tile([128, 4], f32)
            ident = wpool.tile([128, 128], f16)
            for t_, d_ in ((wth, wth_d), (wph, wph_d), (wg, wg_d),
                           (wz, wz_d), (bth, bth_d), (bph, bph_d),
                           (bzp, bzp_d), (gam, gam_d), (bet, bet_d),
                           (ident, id_d)):
                nc.sync.dma_start(t_[:], d_[:])

            # ------- persistent activations -------
            phi = phip.tile([128, 2, T], f16)        # [ci_p, m, s]
            th = thp.tile([128, 2, NQ], f16)         # [ci_p, m, q]
            gt = gtp.tile([128, T // 128, CI], f16)  # [s_p, j, ci]
            wy = wyp.tile([128, 4, NQ], f32)         # [c_p, cc, q]

            # ------- conv phase: phi + g from x (keys) -------
            g_sb = [ppool.tile([128, T], f16, tag="P", name=f"g_sb{m}")
                    for m in range(2)]
            for tb in range(NTB):
                xt = xpool.tile([128, 4, 512], f16, tag="xt")
                nc.sync.dma_start(xt[:], x_d[:, :, tb * 512:(tb + 1) * 512])
                for m in range(2):
                    ps = mmp.tile([128, 512], f32)
                    for kc in range(4):
                        nc.tensor.matmul(
                            ps[:], wph[:, kc, m * 128:(m + 1) * 128],
                            xt[:, kc, :], start=(kc == 0), stop=(kc == 3))
                    nc.scalar.activation(phi[:, m, tb * 512:(tb + 1) * 512],
                                         ps[:], AF.Identity,
                                         bias=bph[:, m:m + 1])
                for m in range(2):
                    ps = mmp.tile([128, 512], f32)
                    for kc in range(4):
                        nc.tensor.matmul(
                            ps[:], wg[:, kc, m * 128:(m + 1) * 128],
                            xt[:, kc, :], start=(kc == 0), stop=(kc == 3))
                    nc.scalar.activation(g_sb[m][:, tb * 512:(tb + 1) * 512],
                                         ps[:], AF.Identity)
            # transpose g -> gt  [s_p, j, ci]
            for j in range(T // 128):
                for m in range(2):
                    tr = trp.tile([128, 128], f16, tag="tr")
                    nc.tensor.transpose(tr[:], g_sb[m][:, j * 128:(j + 1) * 128],
                                        ident[:])
                    nc.any.tensor_copy(gt[:, j, m * 128:(m + 1) * 128], tr[:])

            # ------- conv phase: theta from xq (queries) -------
            for tb in range(NQB):
                xt = xpool.tile([128, 4, 512], f16, tag="xt")
                nc.sync.dma_start(xt[:], xq_d[:, :, tb * 512:(tb + 1) * 512])
                for m in range(2):
                    ps = mmp.tile([128, 512], f32)
                    for kc in range(4):
                        nc.tensor.matmul(
                            ps[:], wth[:, kc, m * 128:(m + 1) * 128],
                            xt[:, kc, :], start=(kc == 0), stop=(kc == 3))
                    nc.scalar.activation(th[:, m, tb * 512:(tb + 1) * 512],
                                         ps[:], AF.Identity,
                                         bias=bth[:, m:m + 1])

            # ------- attention, software-pipelined in half-groups -------
            s1acc = stp.tile([128, 4, 4], f32)
            s2acc = stp.tile([128, 4, 4], f32)
            P_t = {}      # qt -> P tile

            def softmax_stage(qt):
                P = ppool.tile([128, T], f16, tag="P")
                mx8 = smp.tile([128, 8], f32, tag="mx8")
                l8 = smp.tile([128, 8], f32, tag="l8")
                mh = [None, None]
                for h in range(2):
                    blocks = []
                    for sb in range(4):
                        j = h * 4 + sb
                        ps = mmp.tile([128, 512], f32)
                        for m in range(2):
                            nc.tensor.matmul(
                                ps[:], th[:, m, qt * 128:(qt + 1) * 128],
                                phi[:, m, j * 512:(j + 1) * 512],
                                start=(m == 0), stop=(m == 1))
                        nc.vector.reduce_max(mx8[:, j:j + 1], ps[:], axis=AX.X)
                        blocks.append(ps)
                    mh_h = smp.tile([128, 1], f32, tag="mh")
                    nc.vector.reduce_max(mh_h[:], mx8[:, h * 4:(h + 1) * 4],
                                         axis=AX.X)
                    nm_h = smp.tile([128, 1], f32, tag="nm")
                    nc.vector.tensor_scalar_mul(nm_h[:], mh_h[:], -1.0)
                    mh[h] = mh_h
                    for sb in range(4):
                        j = h * 4 + sb
                        nc.scalar.activation(
                            P[:, j * 512:(j + 1) * 512], blocks[sb][:],
                            AF.Exp, bias=nm_h[:], accum_out=l8[:, j:j + 1])
                # merge the two halves: r_h = exp(m_h - m) / l
                mg = smp.tile([128, 1], f32, tag="mg")
                nc.vector.tensor_max(mg[:], mh[0][:], mh[1][:])
                nmg = smp.tile([128, 1], f32, tag="nmg")
                nc.vector.tensor_scalar_mul(nmg[:], mg[:], -1.0)
                ch = []
                for h in range(2):
                    c_h = smp.tile([128, 1], f32, tag="ch")
                    nc.scalar.activation(c_h[:], mh[h][:], AF.Exp, bias=nmg[:])
                    ch.append(c_h)
                lsum = smp.tile([128, 2], f32, tag="lsum")
                for h in range(2):
                    nc.vector.reduce_sum(lsum[:, h:h + 1],
                                         l8[:, h * 4:(h + 1) * 4], axis=AX.X)
                lt = smp.tile([128, 2], f32, tag="lt")
                for h in range(2):
                    nc.vector.tensor_mul(lt[:, h:h + 1], ch[h][:],
                                         lsum[:, h:h + 1])
                lg = smp.tile([128, 1], f32, tag="lg")
                nc.vector.tensor_add(lg[:], lt[:, 0:1], lt[:, 1:2])
                rg = smp.tile([128, 1], f32, tag="rg")
                nc.vector.reciprocal(rg[:], lg[:])
                for h in range(2):
                    r_h = smp.tile([128, 1], f32, tag="rh")
                    nc.vector.tensor_mul(r_h[:], ch[h][:], rg[:])
                    half = P[:, h * (T // 2):(h + 1) * (T // 2)]
                    nc.vector.tensor_scalar_mul(half, half, r_h[:])
                P_t[qt] = P

            def tr_y_stage(hg):
                qts = (2 * hg, 2 * hg + 1)
                # one PSUM bank per ci accumulation group: a start=True
                # matmul resets has_written for its whole bank, so the two
                # interleaved groups must not share one
                y_ps = [yps.tile([128, 256], f32, name=f"y_ps{ci}",
                                 tag=f"yps{ci}") for ci in range(2)]
                for j in range(T // 128):
                    ptw = ptp.tile([128, 256], f16)
                    for t in range(2):
                        qt = qts[t]
                        tr = trp.tile([128, 128], f16, tag="tr")
                        nc.tensor.transpose(tr[:],
                                            P_t[qt][:, j * 128:(j + 1) * 128],
                                            ident[:])
                        nc.any.tensor_copy(ptw[:, t * 128:(t + 1) * 128],
                                           tr[:])
                    for ci in range(2):
                        nc.tensor.matmul(
                            y_ps[ci][:], gt[:, j, ci * 128:(ci + 1) * 128],
                            ptw[:], start=(j == 0),
                            stop=(j == T // 128 - 1))
                for qt in qts:
                    del P_t[qt]
                return y_ps

            ygrp = None
            for hg in range(NHG + 1):
                if hg < NHG:
                    softmax_stage(2 * hg)
                    softmax_stage(2 * hg + 1)
                if hg >= 1:
                    src = hg - 1
                    if src % 2 == 0:
                        ygrp = ygp.tile([128, 2, 512], f16, tag="ygrp")
                    y_ps = tr_y_stage(src)
                    qoff = (src % 2) * 256
                    for ci in range(2):
                        nc.scalar.activation(
                            ygrp[:, ci, qoff:qoff + 256],
                            y_ps[ci][:], AF.Identity)
                    if src % 2 == 1:
                        g = src // 2
                        for cc in range(4):
                            ps = mmp.tile([128, 512], f32)
                            for m in range(2):
                                nc.tensor.matmul(
                                    ps[:], wz[:, m, cc * 128:(cc + 1) * 128],
                                    ygrp[:, m, :], start=(m == 0),
                                    stop=(m == 1))
                            nc.scalar.activation(
                                wy[:, cc, g * 512:(g + 1) * 512], ps[:],
                                AF.Identity, bias=bzp[:, cc:cc + 1],
                                accum_out=s1acc[:, cc, g:g + 1])
                            sq = sqp.tile([128, 512], f32)
                            nc.scalar.activation(
                                sq[:], wy[:, cc, g * 512:(g + 1) * 512],
                                AF.Square, accum_out=s2acc[:, cc, g:g + 1])

            # ------- BN stats + collective -------
            stats = stp.tile([128, 8], f32)
            nc.vector.reduce_sum(stats[:, 0:4], s1acc[:], axis=AX.X)
            nc.vector.reduce_sum(stats[:, 4:8], s2acc[:], axis=AX.X)
            nc.sync.dma_start(cc_in[:, :], stats[:])
            nc.gpsimd.collective_compute(
                "AllReduce", mybir.AluOpType.add,
                replica_groups=[list(range(N_CORES))],
                ins=[cc_in.ap().opt()], outs=[cc_out.ap().opt()])
            stin = stp.tile([128, 8], f32)
            nc.sync.dma_start(stin[:], cc_out[:, :])
            inv_n = 1.0 / (B * T)
            mean = stp.tile([128, 4], f32)
            nc.vector.tensor_scalar_mul(mean[:], stin[:, 0:4], inv_n)
            ex2 = stp.tile([128, 4], f32)
            nc.vector.tensor_scalar_mul(ex2[:], stin[:, 4:8], inv_n)
            msq = stp.tile([128, 4], f32)
            nc.vector.tensor_mul(msq[:], mean[:], mean[:])
            var = stp.tile([128, 4], f32)
            nc.vector.tensor_sub(var[:], ex2[:], msq[:])
            vpe = stp.tile([128, 4], f32)
            nc.vector.tensor_scalar_add(vpe[:], var[:], BN_EPS)
            inv = stp.tile([128, 4], f32)
            nc.vector.reciprocal(inv[:], vpe[:])
            rstd = stp.tile([128, 4], f32)
            nc.scalar.sqrt(rstd[:], inv[:])
            a_t = stp.tile([128, 4], f32)
            nc.vector.tensor_mul(a_t[:], gam[:], rstd[:])
            ma = stp.tile([128, 4], f32)
            nc.vector.tensor_mul(ma[:], mean[:], a_t[:])
            bsh = stp.tile([128, 4], f32)
            nc.vector.tensor_sub(bsh[:], bet[:], ma[:])

            # ------- BN apply + residual + write out -------
            for cc in range(4):
                for qb in range(NQB):
                    xr = apl.tile([128, 512], f32, tag="xr")
                    nc.sync.dma_start(
                        xr[:], xq32_d[:, cc, qb * 512:(qb + 1) * 512])
                    t1 = apl.tile([128, 512], f32, tag="t1")
                    nc.scalar.activation(
                        t1[:], wy[:, cc, qb * 512:(qb + 1) * 512],
                        AF.Identity, scale=a_t[:, cc:cc + 1],
                        bias=bsh[:, cc:cc + 1])
                    outt = apl.tile([128, 512], f32, tag="outt")
                    nc.vector.tensor_add(outt[:], t1[:], xr[:])
                    nc.sync.dma_start(z_d[:, cc, qb * 512:(qb + 1) * 512],
                                      outt[:])

    nc.compile()
    return nc


def _get_compiled():
    global _COMPILED
    if _COMPILED is None:
        _COMPILED = _build()
    return _COMPILED


def _prep_inputs(x, W_g, b_g, W_theta, b_theta, W_phi, b_phi, W_z, b_z,
                 gamma, beta):
    """Host-side slicing/layout.  Returns list of per-core input dicts."""
    def cmaj16(w):                     # (CI, C) -> [128, C//128, CI] fp16
        return np.ascontiguousarray(
            w.T.reshape(C // 128, 128, w.shape[0]).transpose(1, 0, 2)
        ).astype(np.float16)

    wth = cmaj16(W_theta)
    wph = cmaj16(W_phi)
    wg = cmaj16(W_g)
    wz = np.ascontiguousarray(
        W_z.T.reshape(2, 128, C).transpose(1, 0, 2)).astype(np.float16)
    bth = np.ascontiguousarray(b_theta.reshape(2, 128).T)
    bph = np.ascontiguousarray(b_phi.reshape(2, 128).T)
    bzp = np.ascontiguousarray(
        (b_z.astype(np.float64) +
         W_z.astype(np.float64) @ b_g.astype(np.float64))
        .reshape(4, 128).T).astype(np.float32)
    gam = np.ascontiguousarray(gamma.reshape(4, 128).T)
    bet = np.ascontiguousarray(beta.reshape(4, 128).T)
    ident = np.eye(128, dtype=np.float16)

    in_maps = []
    for k in range(N_CORES):
        b = k // 2
        q0 = (k % 2) * NQ
        xb32 = np.ascontiguousarray(
            x[b].reshape(4, 128, T).transpose(1, 0, 2))   # [128,4,T] f32
        xq32 = np.ascontiguousarray(xb32[:, :, q0:q0 + NQ])
        in_maps.append({
            "x": xb32.astype(np.float16),
            "xq": xq32.astype(np.float16),
            "xq32": xq32,
            "wthT": wth, "wphT": wph, "wgT": wg, "wzT": wz,
            "bth": bth, "bph": bph, "bzp": bzp, "gam": gam, "bet": bet,
            "ident": ident,
        })
    return in_maps


def kernel(x, W_g, b_g, W_theta, b_theta, W_phi, b_phi, W_z, b_z,
           gamma, beta, mesh=None, _trace=False):
    from concourse import bass_utils
    x = np.asarray(x, dtype=np.float32)
    args = [np.asarray(a, dtype=np.float32) for a in
            (W_g, b_g, W_theta, b_theta, W_phi, b_phi, W_z, b_z, gamma, beta)]
    nc = _get_compiled()
    in_maps = _prep_inputs(x, *args)
    res = bass_utils.run_bass_kernel_spmd(
        nc, in_maps, core_ids=list(range(N_CORES)), trace=_trace)
    out = np.empty((B, C, T), dtype=np.float32)
    for k in range(N_CORES):
        b = k // 2
        q0 = (k % 2) * NQ
        zc = res.results[k]["z"]                       # [128, 4, NQ]
        out[b, :, q0:q0 + NQ] = zc.transpose(1, 0, 2).reshape(C, NQ)
    if _trace:
        kernel._last_exec_time_ns = res.exec_time_ns
    return out[..., None]
